# revision 71
# baseline (speedup 1.0000x reference)
"""Causal GQA attention block (B=2, T=2048, C=2048, H=16, HKV=4, D=128, RoPE)
on 8 Trainium2 NeuronCores.

Sharding: core c handles batch b = c//4 and kv-group g = c%4 (4 q heads +
1 kv head per core).  The output projection is row-parallel: each core
produces a partial [T, C] contribution; the host sums the 4 partials per
batch.

Device-side design (v2):
  - Projections and the output GEMM run as fp8e4m3 DoubleRow matmuls with a
    3-term hi+lo decomposition (x_hi*w_hi + x_hi*w_lo + x_lo*w_hi), giving
    ~bf16 accuracy at 0.75x the fp32r PE cost in the TRN2 cost model.
    Operands are pre-scaled by powers of two into fp8 normal range; the
    compensations are folded into the rope tables (2^-8), the v copy scale
    (2^-5) and the softmax-denominator ones-vector (2^2 matching wo's 2^2).
  - v is computed with x as the stationary operand so it lands directly in
    [t, d] layout (no transposes).
  - Scores/PV are causal-exact at 128-column granularity via variable-width
    tiles; q/k in fp16, es/v in bf16.
  - Softmax denominators come from tiny es-stationary matmuls (out free size
    1) accumulated per t-subtile into one PSUM bank; the reciprocal is
    rebuilt as a [1, 512] row via per-partition identity scaling plus a
    rank-1 PE matmul, then partition-broadcast by gpsimd.
  - The three phases are software-pipelined at js-step granularity:
    produce (scores+exp+mask) and consume (dn minis+PV+norm) loops of
    adjacent head-rows interleave, with the next chunk's projections and
    the previous chunk's output GEMM pumped as filler between steps so the
    activation engine's exp latency never stalls the PE.
  - y tiles are staged to SBUF as bf16 and summed across cores on the host
    in f32.

Mode (env BASS_ATTN_MODE): "dr" (default, fp8 DR output GEMM),
"cbf16" (bf16 output GEMM, a bit slower but more accurate).
"""

import os
from contextlib import ExitStack

import numpy as np
import ml_dtypes

import concourse.bass as bass
import concourse.tile as tile
from concourse import bacc, mybir
from concourse.bass_utils import run_bass_kernel_spmd
from concourse.masks import make_identity

# problem constants
B, T, C = 2, 2048, 2048
H, HKV, D = 16, 4, 128
GROUP = H // HKV           # 4 q heads per kv head
THETA = 1000000.0
SCALE = D ** -0.5

P = 128                    # partitions
TCH = 512                  # t-chunk
NJT = T // TCH             # 4
NK = C // P                # 16 contraction tiles of 128
NH = GROUP                 # 4 local q heads
N_CORES = 8

F32 = mybir.dt.float32
F32R = mybir.dt.float32r
F16 = mybir.dt.float16
BF16 = mybir.dt.bfloat16
FP8 = mybir.dt.float8e4
DRM = mybir.MatmulPerfMode.DoubleRow
MUL = mybir.AluOpType.mult
ADD = mybir.AluOpType.add
SUB = mybir.AluOpType.subtract
EXP = mybir.ActivationFunctionType.Exp
CPY = mybir.ActivationFunctionType.Copy

# power-of-two prescales that keep the fp8 operands in normal range
WQ_SC = 2.0 ** 8           # on wq (with SCALE folded); undone by rope tables
WK_SC = 2.0 ** 8           # on wk; undone by rope tables
WV_SC = 2.0 ** 5           # on wv; undone by the v copy scale
WO_SC = 2.0 ** 2           # on wo; undone via the denominator ones value
ONES_VAL = WO_SC           # dn = WO_SC * sum(es)  =>  ot_q = attn / WO_SC


def build_program(mode="dr"):
    """Build and compile the per-core Bass program. Returns nc."""
    c_dr = (mode != "cbf16")

    nc = bacc.Bacc("TRN2", target_bir_lowering=False, debug=False)

    xh_d = nc.dram_tensor("xh", [C, T], FP8, kind="ExternalInput").ap()
    xl_d = nc.dram_tensor("xl", [C, T], FP8, kind="ExternalInput").ap()
    wqh_d = nc.dram_tensor("wqh", [P, NK * NH * D], FP8, kind="ExternalInput").ap()
    wql_d = nc.dram_tensor("wql", [P, NH * NK * D], FP8, kind="ExternalInput").ap()
    wkh_d = nc.dram_tensor("wkh", [P, NK * D], FP8, kind="ExternalInput").ap()
    wkl_d = nc.dram_tensor("wkl", [P, NK * D], FP8, kind="ExternalInput").ap()
    wvh_d = nc.dram_tensor("wvh", [P, NK * D], FP8, kind="ExternalInput").ap()
    wvl_d = nc.dram_tensor("wvl", [P, NK * D], FP8, kind="ExternalInput").ap()
    wo_dt = FP8 if c_dr else BF16
    woh_d = nc.dram_tensor("woh", [P, NH * C], wo_dt, kind="ExternalInput").ap()
    wol_d = nc.dram_tensor("wol", [P, NH * C], wo_dt, kind="ExternalInput").ap()
    cos_d = nc.dram_tensor("cosT", [P, T], F16, kind="ExternalInput").ap()
    sin_d = nc.dram_tensor("sinT", [P, T], F16, kind="ExternalInput").ap()
    tri_d = nc.dram_tensor("tri", [P, P], BF16, kind="ExternalInput").ap()
    ones_d = nc.dram_tensor("ones", [P, 1], BF16, kind="ExternalInput").ap()
    onec_d = nc.dram_tensor("onec", [P, 1], F32R, kind="ExternalInput").ap()
    y_d = nc.dram_tensor("y", [T, C], BF16, kind="ExternalOutput").ap()

    with tile.TileContext(nc) as tc, ExitStack() as ctx:
        wp = ctx.enter_context(tc.tile_pool(name="w", bufs=1))
        xp = ctx.enter_context(tc.tile_pool(name="x", bufs=2))
        st = ctx.enter_context(tc.tile_pool(name="st", bufs=1))
        qp = ctx.enter_context(tc.tile_pool(name="qp", bufs=2))
        op = ctx.enter_context(tc.tile_pool(name="op", bufs=4))
        esp = ctx.enter_context(tc.tile_pool(name="es", bufs=32))
        rp = ctx.enter_context(tc.tile_pool(name="rp", bufs=3))
        paA = ctx.enter_context(tc.tile_pool(name="paA", bufs=2, space="PSUM"))
        psS = ctx.enter_context(tc.tile_pool(name="psS", bufs=3, space="PSUM"))
        psOY = ctx.enter_context(tc.tile_pool(name="psOY", bufs=2, space="PSUM"))
        psD = ctx.enter_context(tc.tile_pool(name="psD", bufs=1, space="PSUM"))

        # ---- weight/table/x loads, all on the scalar queue in need order ----
        # (single queue => DMA engines serve in true priority order; weights
        # arrive pre-shuffled to [P, NK*cols] so every row is a >=512B run)
        wkh = wp.tile([P, NK, D], FP8, tag="wkh")
        wkl = wp.tile([P, NK, D], FP8, tag="wkl")
        wvh = wp.tile([P, NK, D], FP8, tag="wvh")
        wvl = wp.tile([P, NK, D], FP8, tag="wvl")
        for hf in range(2):
            ks = slice(hf * 8, (hf + 1) * 8)
            csl = slice(hf * 8 * D, (hf + 1) * 8 * D)
            nc.sync.dma_start(wkh[:, ks, :],
                              wkh_d[:, csl].rearrange("p (ko o) -> p ko o", ko=8))
            nc.sync.dma_start(wkl[:, ks, :],
                              wkl_d[:, csl].rearrange("p (ko o) -> p ko o", ko=8))
        wqh = wp.tile([P, NK, NH * D], FP8, tag="wqh")
        wql = wp.tile([P, NH, NK, D], FP8, tag="wql")
        woh = wp.tile([P, NH, C], wo_dt, tag="woh")
        wol = wp.tile([P, NH, C], wo_dt, tag="wol")
        ident_sb = wp.tile([P, P], F32, tag="ident")
        make_identity(nc, ident_sb[:])

        # ---- persistent state ----
        krot = st.tile([P, T], F16, tag="krot")
        v_sb = st.tile([P, T // P, D], BF16, tag="v")
        ot_dt = FP8 if c_dr else BF16

        # ---- phase A emitters ----
        def load_x(jt):
            xh_t = xp.tile([P, NK, TCH], FP8, tag="xh", name=f"xh{jt}")
            xl_t = xp.tile([P, NK, TCH], FP8, tag="xl", name=f"xl{jt}")
            tsl = slice(jt * TCH, (jt + 1) * TCH)
            nc.scalar.dma_start(
                xh_t[:], xh_d[:, tsl].rearrange("(ko p) t -> p ko t", p=P))
            nc.scalar.dma_start(
                xl_t[:], xl_d[:, tsl].rearrange("(ko p) t -> p ko t", p=P))
            return xh_t, xl_t

        def load_x0_half(xts, hf, split=1):
            xh_t, xl_t = xts
            nq = 8 // split
            for q in range(split):
                k0 = hf * 8 + q * nq
                ks = slice(k0, k0 + nq)
                rows = slice(k0 * P, (k0 + nq) * P)
                tsl = slice(0, TCH)
                nc.scalar.dma_start(
                    xh_t[:, ks, :],
                    xh_d[rows, tsl].rearrange("(ko p) t -> p ko t", p=P))
                nc.scalar.dma_start(
                    xl_t[:, ks, :],
                    xl_d[rows, tsl].rearrange("(ko p) t -> p ko t", p=P))

        def load_rest_of_weights():
            cos_sb = wp.tile([P, T], F16, tag="cos")
            sin_sb = wp.tile([P, T], F16, tag="sin")
            nc.scalar.dma_start(cos_sb[:, 0:TCH], cos_d[:, 0:TCH])
            nc.scalar.dma_start(sin_sb[:, 0:TCH], sin_d[:, 0:TCH])
            tri_sb = wp.tile([P, P], BF16, tag="tri")
            nc.scalar.dma_start(tri_sb[:], tri_d)
            ones_sb = wp.tile([P, 1], BF16, tag="ones")
            nc.scalar.dma_start(ones_sb[:], ones_d)
            onec_sb = wp.tile([P, 1], F32R, tag="onec")
            nc.scalar.dma_start(onec_sb[:], onec_d)
            for o in range(1, NH):
                load_wql_head(o)
            nc.scalar.dma_start(wvh[:], wvh_d.rearrange("p (ko o) -> p ko o", ko=NK))
            nc.scalar.dma_start(wvl[:], wvl_d.rearrange("p (ko o) -> p ko o", ko=NK))
            return cos_sb, sin_sb, tri_sb, ones_sb, onec_sb

        def load_tables_rest():
            nc.scalar.dma_start(cos_sb[:, TCH:T], cos_d[:, TCH:T])
            nc.scalar.dma_start(sin_sb[:, TCH:T], sin_d[:, TCH:T])

        def load_wq_half(hf):
            ks = slice(hf * 8, (hf + 1) * 8)
            csl = slice(hf * 8 * D * NH, (hf + 1) * 8 * D * NH)
            nc.scalar.dma_start(
                wqh[:, ks, :], wqh_d[:, csl].rearrange("p (ko o) -> p ko o", ko=8))

        def load_wql_head(o):
            csl = slice(o * NK * D, (o + 1) * NK * D)
            nc.scalar.dma_start(
                wql[:, o, :, :],
                wql_d[:, csl].rearrange("p (ko o) -> p ko o", ko=NK))

        def load_wo():
            nc.scalar.dma_start(woh[:], woh_d.rearrange("p (h c) -> p h c", h=NH))
            if c_dr:
                nc.scalar.dma_start(wol[:], wol_d.rearrange("p (h c) -> p h c", h=NH))

        def a_filler(jt, xts, qrot_t):
            """Generator emitting A(jt); yields ~every 4 PE matmuls."""
            xh_t, xl_t = xts

            def qk_chain(o):
                acc = paA.tile([P, TCH], F32, tag="pa", name=f"aqk{jt}_{o}")
                wh = wqh if o < NH else wkh
                osl = slice(o * D, (o + 1) * D) if o < NH else slice(0, D)
                for hf in range(2):
                    tsl = slice(hf * 256, (hf + 1) * 256)
                    ar = acc[:, tsl]
                    n = 0
                    for term in range(3):
                        for j2 in range(NK // 2):
                            ksl = slice(2 * j2, 2 * j2 + 2)
                            if term == 0:
                                xa, wa = xh_t[:, ksl, tsl], wh[:, ksl, osl]
                            elif term == 1:
                                xa, wa = xl_t[:, ksl, tsl], wh[:, ksl, osl]
                            elif o < NH:
                                xa, wa = xh_t[:, ksl, tsl], wql[:, o, ksl, :]
                            else:
                                xa, wa = xh_t[:, ksl, tsl], wkl[:, ksl, osl]
                            nc.tensor.matmul(
                                ar, wa, xa,
                                start=(n == 0 and hf == 0), stop=(n == 23),
                                perf_mode=DRM, skip_group_check=(hf == 1))
                            n += 1
                            if n % 4 == 0:
                                yield
                # rope (full 512 chunk): out = acc*cos + swap_halves(acc)*sin
                csl = slice(jt * TCH, (jt + 1) * TCH)
                m1 = rp.tile([P, TCH], F32, tag="m1", name="m1")
                nc.vector.tensor_tensor(m1[:], acc[:], cos_sb[:, csl], MUL)
                m2 = rp.tile([P, TCH], F32, tag="m2", name="m2")
                nc.vector.tensor_tensor(m2[0:64, :], acc[64:128, :],
                                        sin_sb[0:64, csl], MUL)
                nc.vector.tensor_tensor(m2[64:128, :], acc[0:64, :],
                                        sin_sb[64:128, csl], MUL)
                out = qrot_t[:, o, :] if o < NH else krot[:, csl]
                nc.vector.tensor_tensor(out, m1[:], m2[:], ADD)
                yield

            def v_chains():
                vacc = paA.tile([P, TCH], F32, tag="pa", name=f"av{jt}")
                for m in range(4):
                    vr = vacc[:, m * P:(m + 1) * P]
                    tloc = slice(m * P, (m + 1) * P)
                    n = 0
                    for xt, wt in ((xh_t, wvh), (xl_t, wvh), (xh_t, wvl)):
                        for j2 in range(NK // 2):
                            ksl = slice(2 * j2, 2 * j2 + 2)
                            nc.tensor.matmul(
                                vr, xt[:, ksl, tloc], wt[:, ksl, 0:D],
                                start=(n == 0 and m == 0), stop=(n == 23),
                                perf_mode=DRM, skip_group_check=(m > 0))
                            n += 1
                            if n % 4 == 0:
                                yield
                    nc.scalar.activation(v_sb[:, jt * 4 + m, :], vr, CPY,
                                         scale=1.0 / WV_SC)
                    yield

            yield from qk_chain(NH)       # k first
            for o in range(NH):
                yield from qk_chain(o)
            yield from v_chains()

        # ---- phase C filler ----
        def c_filler(jt, ot_hi, ot_lo, final=False):
            """Generator emitting C(jt); yields ~every 6 DR matmuls."""
            for ts in range(4):
                tt = 4 * jt + ts
                for jc in range(NJT):
                    if final:
                        fp = [psS, psOY, paA][(4 * ts + jc) % 3]
                        ftag = ["s", "oy", "pa"][(4 * ts + jc) % 3]
                        yt = fp.tile([P, TCH], F32, tag=ftag,
                                     name=f"y{tt}_{jc}")
                    else:
                        yt = psS.tile([P, TCH], F32, tag="s",
                                      name=f"y{tt}_{jc}")
                    if c_dr:
                        for nf in range(2):
                            yr = yt[:, nf * 256:(nf + 1) * 256]
                            csl = slice(jc * TCH + nf * 256,
                                        jc * TCH + (nf + 1) * 256)
                            n = 0
                            for hp in (0, 2):
                                hsl = slice(hp, hp + 2)
                                tsl = slice(ts * P, (ts + 1) * P)
                                for lt, rt in ((ot_hi, woh), (ot_hi, wol),
                                               (ot_lo, woh)):
                                    nc.tensor.matmul(
                                        yr, lt[:, hsl, tsl], rt[:, hsl, csl],
                                        start=(n == 0 and nf == 0),
                                        stop=(n == 5), perf_mode=DRM,
                                        skip_group_check=(nf == 1))
                                    n += 1
                            yield
                    else:
                        csl = slice(jc * TCH, (jc + 1) * TCH)
                        for h in range(NH):
                            nc.tensor.matmul(
                                yt[:], ot_hi[:, h, ts * P:(ts + 1) * P],
                                woh[:, h, csl], start=(h == 0),
                                stop=(h == NH - 1))
                            if h == 1:
                                yield
                    ys = rp.tile([P, TCH], BF16, tag="ys", bufs=8, name="ys")
                    if final and (4 * ts + jc) % 2 == 1:
                        nc.scalar.copy(ys[:], yt[:])
                    else:
                        nc.vector.tensor_copy(ys[:], yt[:])
                    nc.sync.dma_start(
                        y_d[tt * P:(tt + 1) * P, jc * TCH:(jc + 1) * TCH],
                        ys[:])
                    yield

        # ---- phase B: produce (scores+exp+mask) / consume (dn+PV+norm) ----
        def b_produce(jt, h, qrot_t, rowdata):
            njs = 4 * jt + 4
            for js in range(njs):
                r = max(0, js - 4 * jt)
                w = TCH - P * r
                s_ps = psS.tile([P, w], F32, tag="s", name=f"s{jt}_{h}_{js}")
                nc.tensor.matmul(
                    s_ps[:], krot[:, js * P:(js + 1) * P],
                    qrot_t[:, h, TCH - w:TCH], start=True, stop=True)
                es = esp.tile([P, w], BF16, tag="es", name=f"es{jt}_{h}_{js}")
                nc.scalar.activation(es[:], s_ps[:], EXP)
                if js >= 4 * jt:
                    nc.gpsimd.tensor_tensor(
                        es[:, 0:P], es[:, 0:P], tri_sb[:], MUL)
                rowdata.append((es, js, w, r))
                yield

        def b_consume(jt, h, rowdata, ot_hi, ot_lo):
            njs = 4 * jt + 4
            ot = psOY.tile([P, TCH], F32, tag="oy", name=f"ot{jt}_{h}")
            dnt = psD.tile([P, TCH], F32, tag="dn", name=f"dn{jt}_{h}")
            for idx in range(njs):
                es, js, w, r = rowdata[idx]
                for m in range(w // P):
                    i = r + m
                    nc.tensor.matmul(
                        dnt[:, i:i + 1], es[:, m * P:(m + 1) * P], ones_sb[:],
                        start=(js == 0 and i == 0), stop=(js == 4 * jt + i),
                        skip_group_check=(i > 0))
                nc.tensor.matmul(
                    ot[:, TCH - w:TCH], v_sb[:, js, :], es[:, 0:w],
                    start=(js == 0), stop=(js == njs - 1))
                yield
            # normalization: rec = 1/dn (dn includes the 2^2 from ones).
            # rec sits as [t-sub-pos, sub]; rebuild as a [1, 512] row via
            # identity-scaling + a rank-1 matmul, then broadcast to rb.
            rec = rp.tile([P, 4], F32, tag="rec", name="rec")
            nc.vector.reciprocal(rec[:], dnt[:, 0:4])
            tmq = rp.tile([P, TCH], F32R, tag="tmq", name="tmq")
            for i in range(4):
                nc.vector.tensor_scalar_mul(
                    tmq[:, i * P:(i + 1) * P], ident_sb[:], rec[:, i:i + 1])
            for _ in range(10):
                yield  # let the driver pump other PE work while tmq lands
            drow = psS.tile([1, TCH], F32, tag="s", name=f"drow{jt}_{h}")
            nc.tensor.matmul(drow[:], onec_sb[:], tmq[:], start=True, stop=True)
            drs = rp.tile([1, TCH], F32, tag="drs", name="drs")
            nc.scalar.copy(drs[:], drow[:])
            rbt = rp.tile([P, TCH], F32, tag="rb", name="rb")
            nc.gpsimd.partition_broadcast(rbt[:], drs[:])
            if c_dr:
                mn = rp.tile([P, TCH], F32, tag="mn", name="mn")
                nc.vector.tensor_tensor(mn[:], ot[:], rbt[:], MUL)
                nc.vector.tensor_copy(ot_hi[:, h, :], mn[:])
                nc.vector.tensor_tensor(ot_lo[:, h, :], mn[:],
                                        ot_hi[:, h, :], SUB)
            else:
                nc.vector.tensor_tensor(ot_hi[:, h, :], ot[:], rbt[:], MUL)

        # ---- orchestration: fine-grained software pipeline over jt ----
        def _chain_gens(*gens):
            for g in gens:
                yield from g

        cfill = []          # backlog of C generators (safe to defer)

        cstats = {"played": 0}

        def pump_c(n=1):
            while n > 0 and cfill:
                if next(cfill[0], StopIteration) is StopIteration:
                    cfill.pop(0)
                    continue
                cstats["played"] += 1
                n -= 1

        def drain(gen):
            if gen is not None:
                for _ in gen:
                    pass

        xts = (xp.tile([P, NK, TCH], FP8, tag="xh", name="xh0"),
               xp.tile([P, NK, TCH], FP8, tag="xl", name="xl0"))
        load_x0_half(xts, 0, split=2)
        load_x0_half(xts, 1)
        load_wq_half(0)
        load_wq_half(1)
        load_wql_head(0)
        cos_sb, sin_sb, tri_sb, ones_sb, onec_sb = load_rest_of_weights()
        qrots = {0: qp.tile([P, NH, TCH], F16, tag="qrot", name="qrot0")}
        drain(a_filler(0, xts, qrots[0]))

        for jt in range(NJT):
            afill = None
            if jt < NJT - 1:
                nxts = load_x(jt + 1)
                qrots[jt + 1] = qp.tile([P, NH, TCH], F16, tag="qrot",
                                        name=f"qrot{jt + 1}")
                afill = a_filler(jt + 1, nxts, qrots[jt + 1])
            if jt == 0:
                load_tables_rest()
                load_wo()

            tick = 0

            def pump(n=1):
                nonlocal afill, tick
                while n > 0:
                    tick += 1
                    use_c = cfill and afill is None
                    if use_c:
                        pump_c(1)
                        n -= 1
                        continue
                    if afill is not None:
                        if next(afill, StopIteration) is StopIteration:
                            afill = None
                            continue
                        n -= 1
                        continue
                    n -= 1

            ot_hi = op.tile([P, NH, TCH], ot_dt, tag="oth", name=f"oth{jt}")
            ot_lo = op.tile([P, NH, TCH], ot_dt, tag="otl", name=f"otl{jt}")
            rows = [[] for _ in range(NH)]
            prod = b_produce(jt, 0, qrots[jt], rows[0])
            for _ in prod:
                pump(1)
            for h in range(NH):
                pn = (b_produce(jt, h + 1, qrots[jt], rows[h + 1])
                      if h < NH - 1 else None)
                for _ in b_consume(jt, h, rows[h], ot_hi, ot_lo):
                    if pn is not None:
                        next(pn, None)
                    pump(1)
                if pn is not None:
                    drain(pn)   # no-op unless consume was shorter
            drain(afill)        # qrot(jt+1) must be emitted before stage jt+1
            cfill.append(c_filler(jt, ot_hi, ot_lo, final=(jt == NJT - 1)))

        while cfill:
            pump_c(1)

    nc.compile()
    return nc


def _hilo(a, dt=ml_dtypes.float8_e4m3):
    a = np.ascontiguousarray(a, dtype=np.float32)
    hi = a.astype(dt)
    lo = (a - hi.astype(np.float32)).astype(dt)
    return hi, lo


def host_prep(x, wq, wk, wv, wo, mode="dr"):
    """Build the 8 per-core input maps (numpy, host-side reshuffles only)."""
    c_dr = (mode != "cbf16")
    x = np.asarray(x, dtype=np.float32)
    wq = np.asarray(wq, dtype=np.float32)
    wk = np.asarray(wk, dtype=np.float32)
    wv = np.asarray(wv, dtype=np.float32)
    wo = np.asarray(wo, dtype=np.float32)

    # RoPE even/odd grouping permutation within each head
    perm = np.concatenate([np.arange(0, D, 2), np.arange(1, D, 2)])

    inv_freq = (1.0 / THETA ** (np.arange(0, D, 2, dtype=np.float32) / D)
                ).astype(np.float32)
    pos = np.arange(T, dtype=np.float32)
    freqs = pos[:, None] * inv_freq[None, :]
    cos_t = np.cos(freqs).astype(np.float32).T        # [64, T]
    sin_t = np.sin(freqs).astype(np.float32).T
    cosT = (np.concatenate([cos_t, cos_t], axis=0) / WQ_SC).astype(np.float16)
    sinT = (np.concatenate([-sin_t, sin_t], axis=0) / WQ_SC).astype(np.float16)

    f = np.arange(P)[None, :]
    p = np.arange(P)[:, None]
    tri = (f >= p).astype(ml_dtypes.bfloat16)
    ones = np.full((P, 1), ONES_VAL, dtype=ml_dtypes.bfloat16)
    onec = np.ones((P, 1), dtype=np.float32)

    xh, xl = _hilo(x)                                  # [B, T, C]
    xh = [np.ascontiguousarray(xh[b].T) for b in range(B)]
    xl = [np.ascontiguousarray(xl[b].T) for b in range(B)]

    in_maps = []
    for c in range(N_CORES):
        b, g = divmod(c, GROUP)
        rows = []
        for hh in range(NH):
            h = g * GROUP + hh
            rows.append(wq[h * D + perm, :])
        wq_g = np.concatenate(rows, axis=0) * (SCALE * WQ_SC)   # [512, C]
        wk_g = wk[g * D + perm, :] * WK_SC                      # [128, C]
        wv_g = wv[g * D:(g + 1) * D, :] * WV_SC                 # [128, C]
        wo_g = wo[:, g * NH * D:(g + 1) * NH * D].T * WO_SC     # [512, C]

        def kshuf(wT):
            # [C, cols] -> [P, NK*cols] with row p holding (ko, cols) runs
            cols = wT.shape[1]
            return np.ascontiguousarray(
                wT.reshape(NK, P, cols).transpose(1, 0, 2).reshape(P, NK * cols))

        def hshuf(w):
            # [NH*D, C] -> [P, NH*C] with row p holding (h, C) runs
            return np.ascontiguousarray(
                w.reshape(NH, P, C).transpose(1, 0, 2).reshape(P, NH * C))

        def hqshuf(wT):
            # [C, NH*D] -> [P, NH*NK*D] head-major
            return np.ascontiguousarray(
                wT.reshape(NK, P, NH, D).transpose(1, 2, 0, 3).reshape(P, -1))

        wqT = np.ascontiguousarray(wq_g.T)
        wqh_, _ = _hilo(kshuf(wqT))
        _hi_full = wqT.astype(ml_dtypes.float8_e4m3)
        wql_ = hqshuf(wqT - _hi_full.astype(np.float32)).astype(
            ml_dtypes.float8_e4m3)
        wkh_, wkl_ = _hilo(kshuf(np.ascontiguousarray(wk_g.T)))
        wvh_, wvl_ = _hilo(kshuf(np.ascontiguousarray(wv_g.T)))
        if c_dr:
            woh_, wol_ = _hilo(hshuf(wo_g))
        else:
            # bf16 has the range; keep the WO_SC prescale so the device-side
            # ones/reciprocal compensation stays identical
            woh_ = hshuf(wo_g).astype(ml_dtypes.bfloat16)
            wol_ = np.zeros_like(woh_)

        in_maps.append({
            "xh": xh[b], "xl": xl[b],
            "wqh": wqh_, "wql": wql_,
            "wkh": wkh_, "wkl": wkl_,
            "wvh": wvh_, "wvl": wvl_,
            "woh": woh_, "wol": wol_,
            "cosT": cosT, "sinT": sinT,
            "tri": tri, "ones": ones, "onec": onec,
        })
    return in_maps


_CACHE = {}


def _get_program(mode):
    if mode not in _CACHE:
        _CACHE[mode] = build_program(mode)
    return _CACHE[mode]


def kernel(x, mask, wq, wk, wv, wo):
    mode = os.environ.get("BASS_ATTN_MODE", "dr")
    nc = _get_program(mode)
    in_maps = host_prep(x, wq, wk, wv, wo, mode)
    res = run_bass_kernel_spmd(nc, in_maps, list(range(N_CORES))).results
    out = np.zeros((B, T, C), dtype=np.float32)
    for c in range(N_CORES):
        out[c // GROUP] += np.asarray(res[c]["y"], dtype=np.float32)
    return out


# revision 72
# speedup vs baseline: 1.0043x; 1.0043x over previous
"""Causal GQA attention block (B=2, T=2048, C=2048, H=16, HKV=4, D=128, RoPE)
on 8 Trainium2 NeuronCores.

Sharding: core c handles batch b = c//4 and kv-group g = c%4 (4 q heads +
1 kv head per core).  The output projection is row-parallel: each core
produces a partial [T, C] contribution; the host sums the 4 partials per
batch.

Device-side design (v2):
  - Projections and the output GEMM run as fp8e4m3 DoubleRow matmuls with a
    3-term hi+lo decomposition (x_hi*w_hi + x_hi*w_lo + x_lo*w_hi), giving
    ~bf16 accuracy at 0.75x the fp32r PE cost in the TRN2 cost model.
    Operands are pre-scaled by powers of two into fp8 normal range; the
    compensations are folded into the rope tables (2^-8), the v copy scale
    (2^-5) and the softmax-denominator ones-vector (2^2 matching wo's 2^2).
  - v is computed with x as the stationary operand so it lands directly in
    [t, d] layout (no transposes).
  - Scores/PV are causal-exact at 128-column granularity via variable-width
    tiles; q/k in fp16, es/v in bf16.
  - Softmax denominators come from tiny es-stationary matmuls (out free size
    1) accumulated per t-subtile into one PSUM bank; the reciprocal is
    rebuilt as a [1, 512] row via per-partition identity scaling plus a
    rank-1 PE matmul, then partition-broadcast by gpsimd.
  - The three phases are software-pipelined at js-step granularity:
    produce (scores+exp+mask) and consume (dn minis+PV+norm) loops of
    adjacent head-rows interleave, with the next chunk's projections and
    the previous chunk's output GEMM pumped as filler between steps so the
    activation engine's exp latency never stalls the PE.
  - y tiles are staged to SBUF as bf16 and summed across cores on the host
    in f32.

Mode (env BASS_ATTN_MODE): "dr" (default, fp8 DR output GEMM),
"cbf16" (bf16 output GEMM, a bit slower but more accurate).
"""

import os
from contextlib import ExitStack

import numpy as np
import ml_dtypes

import concourse.bass as bass
import concourse.tile as tile
from concourse import bacc, mybir
from concourse.bass_utils import run_bass_kernel_spmd
from concourse.masks import make_identity

# problem constants
B, T, C = 2, 2048, 2048
H, HKV, D = 16, 4, 128
GROUP = H // HKV           # 4 q heads per kv head
THETA = 1000000.0
SCALE = D ** -0.5

P = 128                    # partitions
TCH = 512                  # t-chunk
NJT = T // TCH             # 4
NK = C // P                # 16 contraction tiles of 128
NH = GROUP                 # 4 local q heads
N_CORES = 8

F32 = mybir.dt.float32
F32R = mybir.dt.float32r
F16 = mybir.dt.float16
BF16 = mybir.dt.bfloat16
FP8 = mybir.dt.float8e4
DRM = mybir.MatmulPerfMode.DoubleRow
MUL = mybir.AluOpType.mult
ADD = mybir.AluOpType.add
SUB = mybir.AluOpType.subtract
EXP = mybir.ActivationFunctionType.Exp
CPY = mybir.ActivationFunctionType.Copy

# power-of-two prescales that keep the fp8 operands in normal range
WQ_SC = 2.0 ** 8           # on wq (with SCALE folded); undone by rope tables
WK_SC = 2.0 ** 8           # on wk; undone by rope tables
WV_SC = 2.0 ** 5           # on wv; undone by the v copy scale
WO_SC = 2.0 ** 2           # on wo; undone via the denominator ones value
ONES_VAL = WO_SC           # dn = WO_SC * sum(es)  =>  ot_q = attn / WO_SC


def build_program(mode="dr"):
    """Build and compile the per-core Bass program. Returns nc."""
    c_dr = (mode != "cbf16")

    nc = bacc.Bacc("TRN2", target_bir_lowering=False, debug=False)

    xh_d = nc.dram_tensor("xh", [C, T], FP8, kind="ExternalInput").ap()
    xl_d = nc.dram_tensor("xl", [C, T], FP8, kind="ExternalInput").ap()
    wqh_d = nc.dram_tensor("wqh", [P, NK * NH * D], FP8, kind="ExternalInput").ap()
    wql_d = nc.dram_tensor("wql", [P, NH * NK * D], FP8, kind="ExternalInput").ap()
    wkh_d = nc.dram_tensor("wkh", [P, NK * D], FP8, kind="ExternalInput").ap()
    wkl_d = nc.dram_tensor("wkl", [P, NK * D], FP8, kind="ExternalInput").ap()
    wvh_d = nc.dram_tensor("wvh", [P, NK * D], FP8, kind="ExternalInput").ap()
    wvl_d = nc.dram_tensor("wvl", [P, NK * D], FP8, kind="ExternalInput").ap()
    wo_dt = FP8 if c_dr else BF16
    woh_d = nc.dram_tensor("woh", [P, NH * C], wo_dt, kind="ExternalInput").ap()
    wol_d = nc.dram_tensor("wol", [P, NH * C], wo_dt, kind="ExternalInput").ap()
    cos_d = nc.dram_tensor("cosT", [P, T], F16, kind="ExternalInput").ap()
    sin_d = nc.dram_tensor("sinT", [P, T], F16, kind="ExternalInput").ap()
    tri_d = nc.dram_tensor("tri", [P, P], BF16, kind="ExternalInput").ap()
    ones_d = nc.dram_tensor("ones", [P, 1], BF16, kind="ExternalInput").ap()
    onec_d = nc.dram_tensor("onec", [P, 1], F32R, kind="ExternalInput").ap()
    y_d = nc.dram_tensor("y", [T, C], BF16, kind="ExternalOutput").ap()

    with tile.TileContext(nc) as tc, ExitStack() as ctx:
        wp = ctx.enter_context(tc.tile_pool(name="w", bufs=1))
        xp = ctx.enter_context(tc.tile_pool(name="x", bufs=2))
        st = ctx.enter_context(tc.tile_pool(name="st", bufs=1))
        qp = ctx.enter_context(tc.tile_pool(name="qp", bufs=2))
        op = ctx.enter_context(tc.tile_pool(name="op", bufs=4))
        esp = ctx.enter_context(tc.tile_pool(name="es", bufs=32))
        rp = ctx.enter_context(tc.tile_pool(name="rp", bufs=3))
        paA = ctx.enter_context(tc.tile_pool(name="paA", bufs=2, space="PSUM"))
        psS = ctx.enter_context(tc.tile_pool(name="psS", bufs=3, space="PSUM"))
        psOY = ctx.enter_context(tc.tile_pool(name="psOY", bufs=2, space="PSUM"))
        psD = ctx.enter_context(tc.tile_pool(name="psD", bufs=1, space="PSUM"))

        # ---- weight/table/x loads, all on the scalar queue in need order ----
        # (single queue => DMA engines serve in true priority order; weights
        # arrive pre-shuffled to [P, NK*cols] so every row is a >=512B run)
        wkh = wp.tile([P, NK, D], FP8, tag="wkh")
        wkl = wp.tile([P, NK, D], FP8, tag="wkl")
        wvh = wp.tile([P, NK, D], FP8, tag="wvh")
        wvl = wp.tile([P, NK, D], FP8, tag="wvl")
        nc.sync.dma_start(wkh[:], wkh_d.rearrange("p (ko o) -> p ko o", ko=NK))
        nc.sync.dma_start(wkl[:], wkl_d.rearrange("p (ko o) -> p ko o", ko=NK))
        wqh = wp.tile([P, NK, NH * D], FP8, tag="wqh")
        wql = wp.tile([P, NH, NK, D], FP8, tag="wql")
        woh = wp.tile([P, NH, C], wo_dt, tag="woh")
        wol = wp.tile([P, NH, C], wo_dt, tag="wol")
        ident_sb = wp.tile([P, P], F32, tag="ident")
        make_identity(nc, ident_sb[:])

        # ---- persistent state ----
        krot = st.tile([P, T], F16, tag="krot")
        v_sb = st.tile([P, T // P, D], BF16, tag="v")
        ot_dt = FP8 if c_dr else BF16

        # ---- phase A emitters ----
        def load_x(jt):
            xh_t = xp.tile([P, NK, TCH], FP8, tag="xh", name=f"xh{jt}")
            xl_t = xp.tile([P, NK, TCH], FP8, tag="xl", name=f"xl{jt}")
            tsl = slice(jt * TCH, (jt + 1) * TCH)
            nc.scalar.dma_start(
                xh_t[:], xh_d[:, tsl].rearrange("(ko p) t -> p ko t", p=P))
            nc.scalar.dma_start(
                xl_t[:], xl_d[:, tsl].rearrange("(ko p) t -> p ko t", p=P))
            return xh_t, xl_t

        def load_x0_half(xts, hf, split=1):
            xh_t, xl_t = xts
            nq = 8 // split
            for q in range(split):
                k0 = hf * 8 + q * nq
                ks = slice(k0, k0 + nq)
                rows = slice(k0 * P, (k0 + nq) * P)
                tsl = slice(0, TCH)
                nc.scalar.dma_start(
                    xh_t[:, ks, :],
                    xh_d[rows, tsl].rearrange("(ko p) t -> p ko t", p=P))
                nc.scalar.dma_start(
                    xl_t[:, ks, :],
                    xl_d[rows, tsl].rearrange("(ko p) t -> p ko t", p=P))

        def load_rest_of_weights():
            cos_sb = wp.tile([P, T], F16, tag="cos")
            sin_sb = wp.tile([P, T], F16, tag="sin")
            nc.scalar.dma_start(cos_sb[:, 0:TCH], cos_d[:, 0:TCH])
            nc.scalar.dma_start(sin_sb[:, 0:TCH], sin_d[:, 0:TCH])
            tri_sb = wp.tile([P, P], BF16, tag="tri")
            nc.scalar.dma_start(tri_sb[:], tri_d)
            ones_sb = wp.tile([P, 1], BF16, tag="ones")
            nc.scalar.dma_start(ones_sb[:], ones_d)
            onec_sb = wp.tile([P, 1], F32R, tag="onec")
            nc.scalar.dma_start(onec_sb[:], onec_d)
            for o in range(1, NH):
                load_wql_head(o)
            nc.scalar.dma_start(wvh[:], wvh_d.rearrange("p (ko o) -> p ko o", ko=NK))
            nc.scalar.dma_start(wvl[:], wvl_d.rearrange("p (ko o) -> p ko o", ko=NK))
            return cos_sb, sin_sb, tri_sb, ones_sb, onec_sb

        def load_tables_rest():
            nc.scalar.dma_start(cos_sb[:, TCH:T], cos_d[:, TCH:T])
            nc.scalar.dma_start(sin_sb[:, TCH:T], sin_d[:, TCH:T])

        def load_wq_half(hf):
            ks = slice(hf * 8, (hf + 1) * 8)
            csl = slice(hf * 8 * D * NH, (hf + 1) * 8 * D * NH)
            nc.scalar.dma_start(
                wqh[:, ks, :], wqh_d[:, csl].rearrange("p (ko o) -> p ko o", ko=8))

        def load_wql_head(o):
            csl = slice(o * NK * D, (o + 1) * NK * D)
            nc.scalar.dma_start(
                wql[:, o, :, :],
                wql_d[:, csl].rearrange("p (ko o) -> p ko o", ko=NK))

        def load_wo():
            nc.scalar.dma_start(woh[:], woh_d.rearrange("p (h c) -> p h c", h=NH))
            if c_dr:
                nc.scalar.dma_start(wol[:], wol_d.rearrange("p (h c) -> p h c", h=NH))

        def a_filler(jt, xts, qrot_t):
            """Generator emitting A(jt); yields ~every 4 PE matmuls."""
            xh_t, xl_t = xts

            def qk_chain(o):
                acc = paA.tile([P, TCH], F32, tag="pa", name=f"aqk{jt}_{o}")
                wh = wqh if o < NH else wkh
                osl = slice(o * D, (o + 1) * D) if o < NH else slice(0, D)
                for hf in range(2):
                    tsl = slice(hf * 256, (hf + 1) * 256)
                    ar = acc[:, tsl]
                    n = 0
                    for term in range(3):
                        for j2 in range(NK // 2):
                            ksl = slice(2 * j2, 2 * j2 + 2)
                            if term == 0:
                                xa, wa = xh_t[:, ksl, tsl], wh[:, ksl, osl]
                            elif term == 1:
                                xa, wa = xl_t[:, ksl, tsl], wh[:, ksl, osl]
                            elif o < NH:
                                xa, wa = xh_t[:, ksl, tsl], wql[:, o, ksl, :]
                            else:
                                xa, wa = xh_t[:, ksl, tsl], wkl[:, ksl, osl]
                            nc.tensor.matmul(
                                ar, wa, xa,
                                start=(n == 0 and hf == 0), stop=(n == 23),
                                perf_mode=DRM, skip_group_check=(hf == 1))
                            n += 1
                            if n % 4 == 0:
                                yield
                # rope (full 512 chunk): out = acc*cos + swap_halves(acc)*sin
                csl = slice(jt * TCH, (jt + 1) * TCH)
                m1 = rp.tile([P, TCH], F32, tag="m1", name="m1")
                nc.vector.tensor_tensor(m1[:], acc[:], cos_sb[:, csl], MUL)
                m2 = rp.tile([P, TCH], F32, tag="m2", name="m2")
                nc.vector.tensor_tensor(m2[0:64, :], acc[64:128, :],
                                        sin_sb[0:64, csl], MUL)
                nc.vector.tensor_tensor(m2[64:128, :], acc[0:64, :],
                                        sin_sb[64:128, csl], MUL)
                out = qrot_t[:, o, :] if o < NH else krot[:, csl]
                nc.vector.tensor_tensor(out, m1[:], m2[:], ADD)
                yield

            def v_chains():
                vacc = paA.tile([P, TCH], F32, tag="pa", name=f"av{jt}")
                for m in range(4):
                    vr = vacc[:, m * P:(m + 1) * P]
                    tloc = slice(m * P, (m + 1) * P)
                    n = 0
                    for xt, wt in ((xh_t, wvh), (xl_t, wvh), (xh_t, wvl)):
                        for j2 in range(NK // 2):
                            ksl = slice(2 * j2, 2 * j2 + 2)
                            nc.tensor.matmul(
                                vr, xt[:, ksl, tloc], wt[:, ksl, 0:D],
                                start=(n == 0 and m == 0), stop=(n == 23),
                                perf_mode=DRM, skip_group_check=(m > 0))
                            n += 1
                            if n % 4 == 0:
                                yield
                    nc.scalar.activation(v_sb[:, jt * 4 + m, :], vr, CPY,
                                         scale=1.0 / WV_SC)
                    yield

            yield from qk_chain(NH)       # k first
            for o in range(NH):
                yield from qk_chain(o)
            yield from v_chains()

        # ---- phase C filler ----
        def c_filler(jt, ot_hi, ot_lo, final=False):
            """Generator emitting C(jt); yields ~every 6 DR matmuls."""
            for ts in range(4):
                tt = 4 * jt + ts
                for jc in range(NJT):
                    if final:
                        fp = [psS, psOY, paA][(4 * ts + jc) % 3]
                        ftag = ["s", "oy", "pa"][(4 * ts + jc) % 3]
                        yt = fp.tile([P, TCH], F32, tag=ftag,
                                     name=f"y{tt}_{jc}")
                    else:
                        yt = psS.tile([P, TCH], F32, tag="s",
                                      name=f"y{tt}_{jc}")
                    if c_dr:
                        for nf in range(2):
                            yr = yt[:, nf * 256:(nf + 1) * 256]
                            csl = slice(jc * TCH + nf * 256,
                                        jc * TCH + (nf + 1) * 256)
                            n = 0
                            for hp in (0, 2):
                                hsl = slice(hp, hp + 2)
                                tsl = slice(ts * P, (ts + 1) * P)
                                for lt, rt in ((ot_hi, woh), (ot_hi, wol),
                                               (ot_lo, woh)):
                                    nc.tensor.matmul(
                                        yr, lt[:, hsl, tsl], rt[:, hsl, csl],
                                        start=(n == 0 and nf == 0),
                                        stop=(n == 5), perf_mode=DRM,
                                        skip_group_check=(nf == 1))
                                    n += 1
                            yield
                    else:
                        csl = slice(jc * TCH, (jc + 1) * TCH)
                        for h in range(NH):
                            nc.tensor.matmul(
                                yt[:], ot_hi[:, h, ts * P:(ts + 1) * P],
                                woh[:, h, csl], start=(h == 0),
                                stop=(h == NH - 1))
                            if h == 1:
                                yield
                    ys = rp.tile([P, TCH], BF16, tag="ys", bufs=8, name="ys")
                    if final and (4 * ts + jc) % 2 == 1:
                        nc.scalar.copy(ys[:], yt[:])
                    else:
                        nc.vector.tensor_copy(ys[:], yt[:])
                    nc.sync.dma_start(
                        y_d[tt * P:(tt + 1) * P, jc * TCH:(jc + 1) * TCH],
                        ys[:])
                    yield

        # ---- phase B: produce (scores+exp+mask) / consume (dn+PV+norm) ----
        def b_produce(jt, h, qrot_t, rowdata):
            njs = 4 * jt + 4
            for js in range(njs):
                r = max(0, js - 4 * jt)
                w = TCH - P * r
                s_ps = psS.tile([P, w], F32, tag="s", name=f"s{jt}_{h}_{js}")
                nc.tensor.matmul(
                    s_ps[:], krot[:, js * P:(js + 1) * P],
                    qrot_t[:, h, TCH - w:TCH], start=True, stop=True)
                es = esp.tile([P, w], BF16, tag="es", name=f"es{jt}_{h}_{js}")
                nc.scalar.activation(es[:], s_ps[:], EXP)
                if js >= 4 * jt:
                    nc.gpsimd.tensor_tensor(
                        es[:, 0:P], es[:, 0:P], tri_sb[:], MUL)
                rowdata.append((es, js, w, r))
                yield

        def b_consume(jt, h, rowdata, ot_hi, ot_lo):
            njs = 4 * jt + 4
            ot = psOY.tile([P, TCH], F32, tag="oy", name=f"ot{jt}_{h}")
            dnt = psD.tile([P, TCH], F32, tag="dn", name=f"dn{jt}_{h}")
            for idx in range(njs):
                es, js, w, r = rowdata[idx]
                for m in range(w // P):
                    i = r + m
                    nc.tensor.matmul(
                        dnt[:, i:i + 1], es[:, m * P:(m + 1) * P], ones_sb[:],
                        start=(js == 0 and i == 0), stop=(js == 4 * jt + i),
                        skip_group_check=(i > 0))
                nc.tensor.matmul(
                    ot[:, TCH - w:TCH], v_sb[:, js, :], es[:, 0:w],
                    start=(js == 0), stop=(js == njs - 1))
                yield
            # normalization: rec = 1/dn (dn includes the 2^2 from ones).
            # rec sits as [t-sub-pos, sub]; rebuild as a [1, 512] row via
            # identity-scaling + a rank-1 matmul, then broadcast to rb.
            rec = rp.tile([P, 4], F32, tag="rec", name="rec")
            nc.vector.reciprocal(rec[:], dnt[:, 0:4])
            tmq = rp.tile([P, TCH], F32R, tag="tmq", name="tmq")
            for i in range(4):
                nc.vector.tensor_scalar_mul(
                    tmq[:, i * P:(i + 1) * P], ident_sb[:], rec[:, i:i + 1])
            for _ in range(10):
                yield  # let the driver pump other PE work while tmq lands
            drow = psS.tile([1, TCH], F32, tag="s", name=f"drow{jt}_{h}")
            nc.tensor.matmul(drow[:], onec_sb[:], tmq[:], start=True, stop=True)
            drs = rp.tile([1, TCH], F32, tag="drs", name="drs")
            nc.scalar.copy(drs[:], drow[:])
            rbt = rp.tile([P, TCH], F32, tag="rb", name="rb")
            nc.gpsimd.partition_broadcast(rbt[:], drs[:])
            if c_dr:
                mn = rp.tile([P, TCH], F32, tag="mn", name="mn")
                nc.vector.tensor_tensor(mn[:], ot[:], rbt[:], MUL)
                nc.vector.tensor_copy(ot_hi[:, h, :], mn[:])
                nc.vector.tensor_tensor(ot_lo[:, h, :], mn[:],
                                        ot_hi[:, h, :], SUB)
            else:
                nc.vector.tensor_tensor(ot_hi[:, h, :], ot[:], rbt[:], MUL)

        # ---- orchestration: fine-grained software pipeline over jt ----
        def _chain_gens(*gens):
            for g in gens:
                yield from g

        cfill = []          # backlog of C generators (safe to defer)

        cstats = {"played": 0}

        def pump_c(n=1):
            while n > 0 and cfill:
                if next(cfill[0], StopIteration) is StopIteration:
                    cfill.pop(0)
                    continue
                cstats["played"] += 1
                n -= 1

        def drain(gen):
            if gen is not None:
                for _ in gen:
                    pass

        xts = (xp.tile([P, NK, TCH], FP8, tag="xh", name="xh0"),
               xp.tile([P, NK, TCH], FP8, tag="xl", name="xl0"))
        load_x0_half(xts, 0)
        load_x0_half(xts, 1)
        load_wq_half(0)
        load_wq_half(1)
        load_wql_head(0)
        cos_sb, sin_sb, tri_sb, ones_sb, onec_sb = load_rest_of_weights()
        qrots = {0: qp.tile([P, NH, TCH], F16, tag="qrot", name="qrot0")}
        drain(a_filler(0, xts, qrots[0]))

        for jt in range(NJT):
            afill = None
            if jt < NJT - 1:
                nxts = load_x(jt + 1)
                qrots[jt + 1] = qp.tile([P, NH, TCH], F16, tag="qrot",
                                        name=f"qrot{jt + 1}")
                afill = a_filler(jt + 1, nxts, qrots[jt + 1])
            if jt == 0:
                load_tables_rest()
                load_wo()

            tick = 0

            def pump(n=1):
                nonlocal afill, tick
                while n > 0:
                    tick += 1
                    use_c = cfill and afill is None
                    if use_c:
                        pump_c(1)
                        n -= 1
                        continue
                    if afill is not None:
                        if next(afill, StopIteration) is StopIteration:
                            afill = None
                            continue
                        n -= 1
                        continue
                    n -= 1

            ot_hi = op.tile([P, NH, TCH], ot_dt, tag="oth", name=f"oth{jt}")
            ot_lo = op.tile([P, NH, TCH], ot_dt, tag="otl", name=f"otl{jt}")
            rows = [[] for _ in range(NH)]
            prod = b_produce(jt, 0, qrots[jt], rows[0])
            for _ in prod:
                pump(1)
            for h in range(NH):
                pn = (b_produce(jt, h + 1, qrots[jt], rows[h + 1])
                      if h < NH - 1 else None)
                for _ in b_consume(jt, h, rows[h], ot_hi, ot_lo):
                    if pn is not None:
                        next(pn, None)
                    pump(1)
                if pn is not None:
                    drain(pn)   # no-op unless consume was shorter
            drain(afill)        # qrot(jt+1) must be emitted before stage jt+1
            cfill.append(c_filler(jt, ot_hi, ot_lo, final=(jt == NJT - 1)))

        while cfill:
            pump_c(1)

    nc.compile()
    return nc


def _hilo(a, dt=ml_dtypes.float8_e4m3):
    a = np.ascontiguousarray(a, dtype=np.float32)
    hi = a.astype(dt)
    lo = (a - hi.astype(np.float32)).astype(dt)
    return hi, lo


def host_prep(x, wq, wk, wv, wo, mode="dr"):
    """Build the 8 per-core input maps (numpy, host-side reshuffles only)."""
    c_dr = (mode != "cbf16")
    x = np.asarray(x, dtype=np.float32)
    wq = np.asarray(wq, dtype=np.float32)
    wk = np.asarray(wk, dtype=np.float32)
    wv = np.asarray(wv, dtype=np.float32)
    wo = np.asarray(wo, dtype=np.float32)

    # RoPE even/odd grouping permutation within each head
    perm = np.concatenate([np.arange(0, D, 2), np.arange(1, D, 2)])

    inv_freq = (1.0 / THETA ** (np.arange(0, D, 2, dtype=np.float32) / D)
                ).astype(np.float32)
    pos = np.arange(T, dtype=np.float32)
    freqs = pos[:, None] * inv_freq[None, :]
    cos_t = np.cos(freqs).astype(np.float32).T        # [64, T]
    sin_t = np.sin(freqs).astype(np.float32).T
    cosT = (np.concatenate([cos_t, cos_t], axis=0) / WQ_SC).astype(np.float16)
    sinT = (np.concatenate([-sin_t, sin_t], axis=0) / WQ_SC).astype(np.float16)

    f = np.arange(P)[None, :]
    p = np.arange(P)[:, None]
    tri = (f >= p).astype(ml_dtypes.bfloat16)
    ones = np.full((P, 1), ONES_VAL, dtype=ml_dtypes.bfloat16)
    onec = np.ones((P, 1), dtype=np.float32)

    xh, xl = _hilo(x)                                  # [B, T, C]
    xh = [np.ascontiguousarray(xh[b].T) for b in range(B)]
    xl = [np.ascontiguousarray(xl[b].T) for b in range(B)]

    in_maps = []
    for c in range(N_CORES):
        b, g = divmod(c, GROUP)
        rows = []
        for hh in range(NH):
            h = g * GROUP + hh
            rows.append(wq[h * D + perm, :])
        wq_g = np.concatenate(rows, axis=0) * (SCALE * WQ_SC)   # [512, C]
        wk_g = wk[g * D + perm, :] * WK_SC                      # [128, C]
        wv_g = wv[g * D:(g + 1) * D, :] * WV_SC                 # [128, C]
        wo_g = wo[:, g * NH * D:(g + 1) * NH * D].T * WO_SC     # [512, C]

        def kshuf(wT):
            # [C, cols] -> [P, NK*cols] with row p holding (ko, cols) runs
            cols = wT.shape[1]
            return np.ascontiguousarray(
                wT.reshape(NK, P, cols).transpose(1, 0, 2).reshape(P, NK * cols))

        def hshuf(w):
            # [NH*D, C] -> [P, NH*C] with row p holding (h, C) runs
            return np.ascontiguousarray(
                w.reshape(NH, P, C).transpose(1, 0, 2).reshape(P, NH * C))

        def hqshuf(wT):
            # [C, NH*D] -> [P, NH*NK*D] head-major
            return np.ascontiguousarray(
                wT.reshape(NK, P, NH, D).transpose(1, 2, 0, 3).reshape(P, -1))

        wqT = np.ascontiguousarray(wq_g.T)
        wqh_, _ = _hilo(kshuf(wqT))
        _hi_full = wqT.astype(ml_dtypes.float8_e4m3)
        wql_ = hqshuf(wqT - _hi_full.astype(np.float32)).astype(
            ml_dtypes.float8_e4m3)
        wkh_, wkl_ = _hilo(kshuf(np.ascontiguousarray(wk_g.T)))
        wvh_, wvl_ = _hilo(kshuf(np.ascontiguousarray(wv_g.T)))
        if c_dr:
            woh_, wol_ = _hilo(hshuf(wo_g))
        else:
            # bf16 has the range; keep the WO_SC prescale so the device-side
            # ones/reciprocal compensation stays identical
            woh_ = hshuf(wo_g).astype(ml_dtypes.bfloat16)
            wol_ = np.zeros_like(woh_)

        in_maps.append({
            "xh": xh[b], "xl": xl[b],
            "wqh": wqh_, "wql": wql_,
            "wkh": wkh_, "wkl": wkl_,
            "wvh": wvh_, "wvl": wvl_,
            "woh": woh_, "wol": wol_,
            "cosT": cosT, "sinT": sinT,
            "tri": tri, "ones": ones, "onec": onec,
        })
    return in_maps


_CACHE = {}


def _get_program(mode):
    if mode not in _CACHE:
        _CACHE[mode] = build_program(mode)
    return _CACHE[mode]


def kernel(x, mask, wq, wk, wv, wo):
    mode = os.environ.get("BASS_ATTN_MODE", "dr")
    nc = _get_program(mode)
    in_maps = host_prep(x, wq, wk, wv, wo, mode)
    res = run_bass_kernel_spmd(nc, in_maps, list(range(N_CORES))).results
    out = np.zeros((B, T, C), dtype=np.float32)
    for c in range(N_CORES):
        out[c // GROUP] += np.asarray(res[c]["y"], dtype=np.float32)
    return out
